# revision 26
# baseline (speedup 1.0000x reference)
"""3-layer GAT on 8 TRN2 NeuronCores.

Strategy (1D vertex-cut, dst-sharded):
  * Nodes are permuted: degree-sorted, dealt round-robin to 8 cores, so each
    core owns a contiguous range of NC=6272 "new" node ids whose windows of
    128 consecutive ids have near-uniform in-degree.
  * Per (core, window) the edge list is laid out as K[w] "slots" x 128 dst
    rows (shared K schedule across cores, padding slots have log_ew=-1e30).
  * Per layer, per window: indirect-DMA gather of [h|als][src] rows (bf16
    tables), attention weight w = exp(leaky(als+ald)+log_ew) (exp rounds to
    bf16 on the ACT write), rhs = [V*w | w], K identity-lhsT matmuls
    accumulate [128, F+H] f32 in PSUM (weighted segment-sum + denominator),
    normalize, bias(+ReLU).
  * Next-layer tables h'=relu(o)@W', als'=o@(W'·a) are produced per window
    (bf16 PE transpose + one matmul) and AllGather'd (bf16) between layers;
    per-window ald' stays resident in SBUF instead of a DRAM round trip.
  * segment_max is omitted: logits are bounded (|logit| < ~8), exp is safe
    in f32, and softmax is shift-invariant, so this is mathematically
    identical to the reference.

HW constraints discovered (do not regress):
  * indirect_dma_start only works with [128, 1] offset columns. Multi-column
    offset APs pass CoreSim but scramble/drop rows on real HW (probed:
    ~C of 128*C rows written), and >=4K descriptors in one instruction
    hangs the SWDGE ring. The per-slot gather loop is therefore load-bearing;
    the ~1us/instruction Q7 descriptor-generation cost (~850 instr/layer)
    is the kernel's critical path.
  * dma_gather (InstDMAGatherAnt) with num_idxs>=2K also hung the NEFF when
    driven from TileContext, and its int16 indices force a two-pass lo/hi
    table split that costs ~40-70% slot padding. Unresolved; a working
    batched gather is the main remaining speedup (~3x on paper).
"""
import numpy as np

# problem constants (hardcoded per harness contract)
N, E, IN, HID, HEADS, OUT = 50000, 800000, 256, 32, 4, 64
SLOPE = 0.2
CORES = 8
P = 128


# ----------------------------------------------------------------------------
# host-side schedule construction (index data only)
# ----------------------------------------------------------------------------
def build_schedule(src, dst, ew, n_nodes, npad, cores):
    """Returns node permutation + per-core slot arrays.

    perm: old->new node id (len npad); Ks: [W] slots per window;
    gidx: [cores, 128, S] int32 src new-ids; logew: [cores, 128, S] f32.
    """
    nc_rows = npad // cores
    wpc = nc_rows // P
    src = np.asarray(src, np.int64)
    dst = np.asarray(dst, np.int64)
    ew = np.asarray(ew, np.float32)

    deg = np.bincount(dst, minlength=npad)
    order = np.argsort(-deg, kind="stable")          # ranks -> old id
    perm = np.empty(npad, np.int64)
    ranks = np.arange(npad)
    perm[order] = (ranks % cores) * nc_rows + ranks // cores

    nsrc = perm[src]
    ndst = perm[dst]
    eorder = np.argsort(ndst, kind="stable")
    nsrc_s = nsrc[eorder].astype(np.int32)
    ndst_s = ndst[eorder]
    ew_s = ew[eorder]

    # split edges of each dst into lo (src < HALF) / hi rectangles so the
    # dma_gather int16 indices stay in range; HALF = npad//2 keeps the split
    # balanced (npad//2 <= 32768 required on both sides).
    HALF = npad // 2
    hi = (nsrc_s >= HALF).astype(np.int64)
    key = ndst_s * 2 + hi
    korder = np.argsort(key, kind="stable")
    nsrc_s = nsrc_s[korder]
    ew_s = ew_s[korder]
    key_s = key[korder]
    ndst_s = ndst_s[korder]
    hi_s = hi[korder]

    counts2 = np.bincount(key_s, minlength=2 * npad)
    starts2 = np.zeros(2 * npad + 1, np.int64)
    np.cumsum(counts2, out=starts2[1:])
    rank2 = np.arange(len(ndst_s)) - starts2[key_s]

    KLs = counts2[0::2].reshape(cores, wpc, P).max(axis=(0, 2)).astype(np.int64)
    KHs = counts2[1::2].reshape(cores, wpc, P).max(axis=(0, 2)).astype(np.int64)
    KLs[(KLs + KHs) == 0] = 1            # >=1 slot so acc/denominator exist
    Ks = KLs + KHs
    offs = np.zeros(wpc + 1, np.int64)
    np.cumsum(Ks, out=offs[1:])
    S = int(offs[-1])

    core_e = ndst_s // nc_rows
    loc = ndst_s % nc_rows
    w_e = loc // P
    p_e = loc % P
    col = offs[w_e] + np.where(hi_s == 1, KLs[w_e], 0) + rank2

    gidx = np.zeros((cores, P, S), np.int64)       # pass-local row ids
    logew = np.full((cores, P, S), -1e30, np.float32)
    flat = (core_e * P + p_e) * S + col
    gidx.reshape(-1)[flat] = nsrc_s - hi_s * HALF
    with np.errstate(divide="ignore"):
        logew.reshape(-1)[flat] = np.log(np.maximum(ew_s, 0.0)).astype(np.float32)

    # int16 wrapped idx layout for dma_gather: per (window, pass), flat index
    # j = k_local*128 + p; tile[r, s] = flat[s*16 + r%16], replicated to 128
    # partitions; columns per window-pass = 8*Kpass, laid out L then H.
    idxg = np.zeros((cores, P, 8 * S), np.int16)
    for w in range(wpc):
        for (k0, kn) in ((0, int(KLs[w])), (int(KLs[w]), int(Ks[w]))):
            if kn == k0:
                continue
            c0, c1 = offs[w] + k0, offs[w] + kn
            blk = gidx[:, :, c0:c1]                   # [cores, P(p), kp]
            fl = blk.transpose(0, 2, 1).reshape(cores, -1)   # j = k*128+p
            wr = fl.reshape(cores, -1, 16).transpose(0, 2, 1)  # [cores,16,8kp]
            idxg[:, :, 8 * c0:8 * c1] = np.tile(wr, (1, 8, 1)).astype(np.int16)
    return perm, Ks, KLs, offs, idxg, logew


def _np_bf16(x):
    import ml_dtypes
    return np.asarray(x, np.float32).astype(ml_dtypes.bfloat16)


# ----------------------------------------------------------------------------
# device program
# ----------------------------------------------------------------------------
def build_program(npad, Ks, offs, S, in_f, hid_heads, out_f, heads3, cores,
                  enable_asserts=False, debug_taps=False):
    KLs = Ks[1]
    Ks = Ks[0]
    """Build the SPMD Bacc program. Shapes:
      xTt   [NT, in_f, 128] bf16   (transposed x, node-tile blocks)
      w1cat [in_f//128, 128, hid_heads+8] bf16
      w2cat [hid_heads, hid_heads+8] bf16
      w3cat [hid_heads, out_f+2] bf16
      b1row/b2row [1, hid_heads] f32 ; b3row [1, out_f] f32
      idxv  [128, S] int32 ; logew [128, S] f32 ; idx_ald [128, WPC] int32
      out   [NC, out_f] f32 (per-core shard)
    """
    import concourse.bacc as bacc
    import concourse.bass as bass
    import concourse.mybir as mybir
    from concourse.masks import make_identity
    from concourse.tile import TileContext

    F32, BF16, I32 = mybir.dt.float32, mybir.dt.bfloat16, mybir.dt.int32
    AF = mybir.ActivationFunctionType
    ALU = mybir.AluOpType

    nc_rows = npad // cores
    wpc = nc_rows // P
    nt = npad // P
    ic = in_f // P                       # input chunk count (2)
    D = hid_heads                         # 128
    H = HEADS
    wmax = int(max(Ks))

    nc = bacc.Bacc("TRN2", target_bir_lowering=False, debug=False,
                   enable_asserts=enable_asserts, num_devices=cores)

    GA = next(g for g in (14, 8, 7, 4, 2, 1) if (npad // P) % g == 0)
    ngrp = (npad // P) // GA
    xTt = nc.dram_tensor("xTt", [ngrp, P, GA * in_f], BF16,
                         kind="ExternalInput")
    w1cat = nc.dram_tensor("w1cat", [ic, P, D + 8], BF16, kind="ExternalInput")
    w2cat = nc.dram_tensor("w2cat", [D, D + 8], BF16, kind="ExternalInput")
    w3cat = nc.dram_tensor("w3cat", [D, out_f + 8], BF16, kind="ExternalInput")
    b1row = nc.dram_tensor("b1row", [1, D], F32, kind="ExternalInput")
    b2row = nc.dram_tensor("b2row", [1, D], F32, kind="ExternalInput")
    b3row = nc.dram_tensor("b3row", [1, out_f], F32, kind="ExternalInput")
    I16 = mybir.dt.int16
    TW = 256                               # bf16 table row (512B, %256B==0)
    HALF = npad // 2
    GCH = 7                                # <=7*128=896 idxs per dma_gather
    idxg_d = nc.dram_tensor("idxg", [P, 8 * S], I16, kind="ExternalInput")
    logew_d = nc.dram_tensor("logew", [P, S], BF16, kind="ExternalInput")
    idx_ald = nc.dram_tensor("idx_ald", [P, wpc], I32, kind="ExternalInput")
    out_d = nc.dram_tensor("out", [nc_rows, out_f], F32, kind="ExternalOutput")

    # internal DRAM
    h1tab = nc.dram_tensor("h1tab", [npad, TW], BF16)
    ald1tab = nc.dram_tensor("ald1tab", [npad, H], BF16)
    ag2h_in = nc.dram_tensor("ag2h_in", [nc_rows, TW], BF16)
    h2tab = nc.dram_tensor("h2tab", [npad, TW], BF16, addr_space="Shared")
    ag3h_in = nc.dram_tensor("ag3h_in", [nc_rows, TW], BF16)
    h3tab = nc.dram_tensor("h3tab", [npad, TW], BF16, addr_space="Shared")

    rg = [list(range(cores))]

    with TileContext(nc) as tc:
        with tc.tile_pool(name="const", bufs=1) as cp, \
             tc.tile_pool(name="xin", bufs=3) as xp, \
             tc.tile_pool(name="work", bufs=3) as wk, \
             tc.tile_pool(name="small", bufs=3) as sm, \
             tc.tile_pool(name="ps", bufs=3, space="PSUM") as pp, \
             tc.tile_pool(name="ps2", bufs=2, space="PSUM") as pp2:

            ident_b = cp.tile([P, P], BF16, tag="identb")
            make_identity(nc, ident_b[:])
            w1_sb = cp.tile([P, ic, D + 8], BF16, tag="w1")
            nc.sync.dma_start(out=w1_sb[:],
                              in_=w1cat.ap().rearrange("c p f -> p c f"))
            w2_sb = cp.tile([P, D + 8], BF16, tag="w2")
            nc.sync.dma_start(out=w2_sb[:], in_=w2cat[:, :])
            w3_sb = cp.tile([P, out_f + 8], BF16, tag="w3")
            nc.sync.dma_start(out=w3_sb[:], in_=w3cat[:, :])
            b1_sb = cp.tile([P, D], F32, tag="b1")
            nc.sync.dma_start(out=b1_sb[:], in_=b1row.ap().to_broadcast((P, D)))
            b2_sb = cp.tile([P, D], F32, tag="b2")
            nc.sync.dma_start(out=b2_sb[:], in_=b2row.ap().to_broadcast((P, D)))
            b3_sb = cp.tile([P, out_f], F32, tag="b3")
            nc.sync.dma_start(out=b3_sb[:],
                              in_=b3row.ap().to_broadcast((P, out_f)))
            idxg_sb = cp.tile([P, 8 * S], I16, tag="idxg")
            nc.sync.dma_start(out=idxg_sb[:], in_=idxg_d[:, :])
            lew_sb = cp.tile([P, S], BF16, tag="lew")
            nc.sync.dma_start(out=lew_sb[:], in_=logew_d[:, :])
            idxa_sb = cp.tile([P, wpc], I32, tag="idxa")
            nc.sync.dma_start(out=idxa_sb[:], in_=idx_ald[:, :])

            # ---------------- stage A: layer-1 tables (full, local) --------
            for t0 in range(0, nt, GA):
                xt = xp.tile([P, GA, ic, P], BF16, tag="xt")
                nc.sync.dma_start(out=xt[:], in_=xTt[t0 // GA])
                h_sb = sm.tile([P, GA, D + H], BF16, tag="hA")
                a_sb = sm.tile([P, GA, H], BF16, tag="aA")
                for g in range(GA):
                    ps = pp.tile([P, D + 8], F32, tag="acc")
                    for c in range(ic):
                        nc.tensor.matmul(ps[:], lhsT=xt[:, g, c, :],
                                         rhs=w1_sb[:, c, :],
                                         start=(c == 0), stop=(c == ic - 1))
                    nc.scalar.activation(h_sb[:, g, :], ps[:, 0:D + H],
                                         AF.Copy)
                    nc.vector.tensor_copy(out=a_sb[:, g, :],
                                          in_=ps[:, D + 4:D + 4 + H])
                nc.sync.dma_start(
                    out=h1tab[t0 * P:(t0 + GA) * P, 0:D + H].rearrange(
                        "(g p) f -> p g f", p=P),
                    in_=h_sb[:])
                nc.sync.dma_start(
                    out=ald1tab[t0 * P:(t0 + GA) * P, :].rearrange(
                        "(g p) f -> p g f", p=P),
                    in_=a_sb[:])

            # gather own ald windows into SBUF (core-dependent rows via
            # data). NOTE: the SWDGE ucode only supports [128,1] offset
            # columns -- multi-column offset APs scramble/drop rows on HW.
            ald1_all = cp.tile([P, wpc, H], BF16, tag="ald1all")
            for w in range(wpc):
                nc.gpsimd.indirect_dma_start(
                    out=ald1_all[:, w, :], out_offset=None, in_=ald1tab[:, :],
                    in_offset=bass.IndirectOffsetOnAxis(
                        ap=idxa_sb[:, w:w + 1], axis=0))
            ald2_all = cp.tile([P, wpc, H], BF16, tag="ald2all")
            ald3_all = cp.tile([P, wpc, 1], BF16, tag="ald3all")

            # ---------------- edge pass ------------------------------------
            def edge_layer(layer, tab, ald_all, F_in, H_l, wnext_sb,
                           F_next, H_n, bias_sb, agh, aldnext):
                """One GAT layer over all windows. ald_all: SBUF tile."""
                for w in range(wpc):
                    K = int(Ks[w])
                    KL = int(KLs[w])
                    off = int(offs[w])
                    r0 = w * P
                    V = wk.tile([P, K, TW], BF16, tag="V")
                    for (p0, p1, base) in ((0, KL, 0), (KL, K, HALF)):
                        for k0 in range(p0, p1, GCH):
                            k1 = min(k0 + GCH, p1)
                            ni = (k1 - k0) * P
                            nc.gpsimd.dma_gather(
                                out_ap=V[:, k0:k1, :],
                                in_ap=tab[base:base + HALF, :],
                                idxs_ap=idxg_sb[:, 8 * (off + k0):
                                                8 * (off + k1)],
                                num_idxs=ni, num_idxs_reg=ni,
                                elem_size=TW)
                    ALS = V[:, :, F_in:F_in + H_l]
                    ald_w = ald_all[:, w, :]
                    # logit = ALS + ald (bcast over K) + logew (bcast over H)
                    logit = wk.tile([P, K, H_l], BF16, tag="logit")
                    ald_b = bass.AP(ald_w.tensor, ald_w.offset,
                                    [ald_w.ap[0], [0, K], [1, H_l]])
                    nc.vector.tensor_add(out=logit[:], in0=ALS, in1=ald_b)
                    lew_ap = lew_sb[:, off:off + K]
                    lew_b = bass.AP(lew_ap.tensor, lew_ap.offset,
                                    [lew_ap.ap[0], [1, K], [0, H_l]])
                    nc.vector.tensor_add(out=logit[:], in0=logit[:],
                                         in1=lew_b)
                    # w = exp(max(0.2*logit, logit)), rounded to bf16 on write
                    wt = wk.tile([P, K, H_l], BF16, tag="wt")
                    nc.vector.scalar_tensor_tensor(
                        out=wt[:], in0=logit[:], scalar=SLOPE, in1=logit[:],
                        op0=ALU.mult, op1=ALU.max)
                    wtb = wk.tile([P, K, H_l], BF16, tag="wtb")
                    nc.scalar.activation(wtb[:], wt[:], AF.Exp)
                    # rhs = [V*w | w]
                    rhs = wk.tile([P, K, F_in + H_l], BF16, tag="rhs")
                    ch = F_in // H_l
                    wrep = bass.AP(wtb.tensor, wtb[:].offset,
                                   [wtb[:].ap[0], [H_l, K], [1, H_l], [0, ch]])
                    nc.vector.tensor_mul(out=rhs[:, :, 0:F_in],
                                         in0=V[:, :, 0:F_in], in1=wrep)
                    nc.vector.tensor_copy(out=rhs[:, :, F_in:F_in + H_l],
                                          in_=wtb[:])
                    acc = pp.tile([P, F_in + H_l], F32, tag="acc")
                    for k in range(K):
                        nc.tensor.matmul(acc[:], lhsT=ident_b[:],
                                         rhs=rhs[:, k, :],
                                         start=(k == 0), stop=(k == K - 1))
                    den = sm.tile([P, H_l], F32, tag="den")
                    nc.vector.tensor_scalar_add(den[:],
                                                acc[:, F_in:F_in + H_l],
                                                1e-16)
                    rec = sm.tile([P, H_l], F32, tag="rec")
                    nc.vector.reciprocal(rec[:], den[:])
                    o = sm.tile([P, F_in], F32, tag="o")
                    rrep = bass.AP(rec.tensor, rec[:].offset,
                                   [rec[:].ap[0], [1, H_l], [0, ch]])
                    nc.vector.tensor_mul(out=o[:], in0=acc[:, 0:F_in],
                                         in1=rrep)
                    nc.vector.tensor_add(out=o[:], in0=o[:], in1=bias_sb[:])
                    if layer < 3:
                        o_b = sm.tile([P, F_in], BF16, tag="ob")
                        nc.scalar.activation(o_b[:], o[:], AF.Relu)
                        # next-layer table rows for this window
                        oT = pp2.tile([P, P], BF16, tag="oT")
                        nc.tensor.transpose(out=oT[:], in_=o_b[:],
                                            identity=ident_b[:])
                        oT_sb = sm.tile([P, P], BF16, tag="oTsb")
                        nc.scalar.activation(oT_sb[:], oT[:], AF.Copy)
                        hn = pp2.tile([P, F_next + 8], F32, tag="hn")
                        nc.tensor.matmul(hn[:], lhsT=oT_sb[:],
                                         rhs=wnext_sb[:, 0:F_next + 8],
                                         start=True, stop=True)
                        hn_sb = sm.tile([P, F_next + H_n], BF16, tag="hnsb")
                        nc.scalar.activation(hn_sb[:, 0:F_next],
                                             hn[:, 0:F_next], AF.Copy)
                        nc.vector.tensor_copy(
                            out=hn_sb[:, F_next:F_next + H_n],
                            in_=hn[:, F_next:F_next + H_n])
                        nc.sync.dma_start(
                            out=agh[r0:r0 + P, 0:F_next + H_n], in_=hn_sb[:])
                        nc.vector.tensor_copy(
                            out=aldnext[:, w, :],
                            in_=hn[:, F_next + 4:F_next + 4 + H_n])
                    else:
                        nc.sync.dma_start(out=out_d[r0:r0 + P, :], in_=o[:])

            # layer 1
            edge_layer(1, h1tab, ald1_all, D, H,
                       w2_sb, D, H, b1_sb, ag2h_in, ald2_all)
            nc.gpsimd.collective_compute(
                "AllGather", mybir.AluOpType.bypass, replica_groups=rg,
                ins=[ag2h_in.ap().opt()], outs=[h2tab.ap().opt()])
            # layer 2
            edge_layer(2, h2tab, ald2_all, D, H,
                       w3_sb, out_f, heads3, b2_sb, ag3h_in, ald3_all)
            nc.gpsimd.collective_compute(
                "AllGather", mybir.AluOpType.bypass, replica_groups=rg,
                ins=[ag3h_in.ap().opt()], outs=[h3tab.ap().opt()])
            # layer 3
            edge_layer(3, h3tab, ald3_all, out_f, heads3,
                       None, 0, 1, b3_sb, None, None)
    nc.finalize()
    return nc


# ----------------------------------------------------------------------------
# host entry point
# ----------------------------------------------------------------------------
def prepare_inputs(x, edge_index, edge_weight, W1, a_src1, a_dst1, b1,
                   W2, a_src2, a_dst2, b2, W3, a_src3, a_dst3, b3,
                   npad, cores):
    """Returns (in_maps, perm, Ks, offs, S)."""
    x = np.asarray(x, np.float32)
    W1 = np.asarray(W1, np.float32)
    W2 = np.asarray(W2, np.float32)
    W3 = np.asarray(W3, np.float32)
    n_nodes, in_f = x.shape
    d1 = W1.shape[1]
    out_f = W3.shape[1]
    heads = np.asarray(a_src1).shape[0]
    hid = d1 // heads

    perm, Ks, KLs, offs, idxg, logew = build_schedule(
        edge_index[0], edge_index[1], edge_weight, n_nodes, npad, cores)

    xp = np.zeros((npad, in_f), np.float32)
    xp[perm[:n_nodes]] = x
    nt = npad // P
    GA = next(g for g in (14, 8, 7, 4, 2, 1) if nt % g == 0)
    ic = in_f // P
    # [ngrp, p, g, c, n] with value xp[(t0+g)*128+n, c*128+p]
    A = xp.reshape(nt // GA, GA, P, ic, P)          # [grp, g, n, c, p]
    xTt = _np_bf16(np.ascontiguousarray(
        A.transpose(0, 4, 1, 3, 2)).reshape(nt // GA, P, GA * in_f))

    def wcat(W, a_s, a_d, h, c):
        wa = (W.reshape(W.shape[0], h, c) * np.asarray(a_s)[None]).sum(-1)
        wd = (W.reshape(W.shape[0], h, c) * np.asarray(a_d)[None]).sum(-1)
        pad = np.zeros((W.shape[0], 4 - wa.shape[1]), np.float32)
        return np.concatenate([W, wa, pad, wd, pad], axis=1)

    w1full = wcat(W1, a_src1, a_dst1, heads, hid)          # [256, 136]
    w1cat = _np_bf16(w1full.reshape(2, P, d1 + 8))
    w2cat = _np_bf16(wcat(W2, a_src2, a_dst2, heads, hid))  # [128, 136]
    w3cat = _np_bf16(wcat(W3, a_src3, a_dst3, 1, out_f))    # [128, 72]

    nc_rows = npad // cores
    wpc = nc_rows // P
    in_maps = []
    for c in range(cores):
        base = c * nc_rows
        ia = (base + np.arange(wpc)[None, :] * P +
              np.arange(P)[:, None]).astype(np.int32)
        in_maps.append(dict(
            xTt=xTt, w1cat=w1cat, w2cat=w2cat, w3cat=w3cat,
            b1row=np.asarray(b1, np.float32).reshape(1, -1),
            b2row=np.asarray(b2, np.float32).reshape(1, -1),
            b3row=np.asarray(b3, np.float32).reshape(1, -1),
            idxg=idxg[c], logew=_np_bf16(logew[c]), idx_ald=ia,
        ))
    return in_maps, perm, np.stack([Ks, KLs]), offs


def kernel(**inputs):
    npad = 50176
    in_maps, perm, Ks, offs = prepare_inputs(
        npad=npad, cores=CORES, **inputs)
    S = int(offs[-1])
    nc = build_program(npad, Ks, offs, S, IN, HEADS * HID, OUT, 1, CORES)

    from concourse.bass_utils import run_bass_kernel_spmd
    res = run_bass_kernel_spmd(nc, in_maps, core_ids=list(range(CORES)))
    shards = [res.results[c]["out"] for c in range(CORES)]
    full = np.concatenate(shards, axis=0)       # [npad, OUT] in new-id order
    return full[perm[:N]].astype(np.float32)



# revision 28
# speedup vs baseline: 1.4373x; 1.4373x over previous
"""3-layer GAT on 8 TRN2 NeuronCores.

Strategy (1D vertex-cut, dst-sharded):
  * Nodes are permuted: degree-sorted, dealt round-robin to 8 cores, so each
    core owns a contiguous range of NC=6272 "new" node ids whose windows of
    128 consecutive ids have near-uniform in-degree.
  * Per (core, window) the edge list is laid out as K[w] "slots" x 128 dst
    rows (shared K schedule across cores, padding slots have log_ew=-1e30).
  * Per layer, per window: indirect-DMA gather of h[src] rows (bf16) and
    als[src] (f32), attention weight w = exp(leaky(als+ald)+log_ew),
    rhs = [V*w | w], K identity-lhsT matmuls accumulate [128, F+H] in PSUM
    (weighted segment-sum + denominator), normalize, bias(+ReLU).
  * Next-layer tables h'=relu(o)@W', als'=o@(W'·a) are produced per window
    (PE transpose + one matmul) and AllGather'd across cores between layers.
  * segment_max is omitted: logits are bounded (|logit| < ~8), exp is safe
    in f32, and softmax is shift-invariant, so this is mathematically
    identical to the reference.
"""
import numpy as np

# problem constants (hardcoded per harness contract)
N, E, IN, HID, HEADS, OUT = 50000, 800000, 256, 32, 4, 64
SLOPE = 0.2
CORES = 8
P = 128


# ----------------------------------------------------------------------------
# host-side schedule construction (index data only)
# ----------------------------------------------------------------------------
def build_schedule(src, dst, ew, n_nodes, npad, cores):
    """Returns node permutation + per-core slot arrays.

    perm: old->new node id (len npad); Ks: [W] slots per window;
    gidx: [cores, 128, S] int32 src new-ids; logew: [cores, 128, S] f32.
    """
    nc_rows = npad // cores
    wpc = nc_rows // P
    src = np.asarray(src, np.int64)
    dst = np.asarray(dst, np.int64)
    ew = np.asarray(ew, np.float32)

    deg = np.bincount(dst, minlength=npad)
    order = np.argsort(-deg, kind="stable")          # ranks -> old id
    perm = np.empty(npad, np.int64)
    ranks = np.arange(npad)
    perm[order] = (ranks % cores) * nc_rows + ranks // cores

    nsrc = perm[src]
    ndst = perm[dst]
    eorder = np.argsort(ndst, kind="stable")
    nsrc_s = nsrc[eorder].astype(np.int32)
    ndst_s = ndst[eorder]
    ew_s = ew[eorder]

    counts = np.bincount(ndst_s, minlength=npad)
    starts = np.zeros(npad + 1, np.int64)
    np.cumsum(counts, out=starts[1:])
    rank_in_dst = np.arange(len(ndst_s)) - starts[ndst_s]

    Ks = counts.reshape(cores, wpc, P).max(axis=(0, 2))
    Ks = np.maximum(Ks, 1).astype(np.int64)
    offs = np.zeros(wpc + 1, np.int64)
    np.cumsum(Ks, out=offs[1:])
    S = int(offs[-1])

    core_e = ndst_s // nc_rows
    loc = ndst_s % nc_rows
    w_e = loc // P
    p_e = loc % P
    col = offs[w_e] + rank_in_dst

    gidx = np.zeros((cores, P, S), np.int32)
    logew = np.full((cores, P, S), -1e30, np.float32)
    flat = (core_e * P + p_e) * S + col
    gidx.reshape(-1)[flat] = nsrc_s
    with np.errstate(divide="ignore"):
        logew.reshape(-1)[flat] = np.log(np.maximum(ew_s, 0.0)).astype(np.float32)
    return perm, Ks, offs, gidx, logew


def _np_bf16(x):
    import ml_dtypes
    return np.asarray(x, np.float32).astype(ml_dtypes.bfloat16)


# ----------------------------------------------------------------------------
# device program
# ----------------------------------------------------------------------------
def build_program(npad, Ks, offs, S, in_f, hid_heads, out_f, heads3, cores,
                  enable_asserts=False, debug_taps=False):
    """Build the SPMD Bacc program. Shapes:
      xTt   [NT, in_f, 128] bf16   (transposed x, node-tile blocks)
      w1cat [in_f//128, 128, hid_heads+8] bf16
      w2cat [hid_heads, hid_heads+8] bf16
      w3cat [hid_heads, out_f+2] bf16
      b1row/b2row [1, hid_heads] f32 ; b3row [1, out_f] f32
      idxv  [128, S] int32 ; logew [128, S] f32 ; idx_ald [128, WPC] int32
      out   [NC, out_f] f32 (per-core shard)
    """
    import concourse.bacc as bacc
    import concourse.bass as bass
    import concourse.mybir as mybir
    from concourse.masks import make_identity
    from concourse.tile import TileContext

    F32, BF16, I32 = mybir.dt.float32, mybir.dt.bfloat16, mybir.dt.int32
    AF = mybir.ActivationFunctionType
    ALU = mybir.AluOpType

    nc_rows = npad // cores
    wpc = nc_rows // P
    nt = npad // P
    ic = in_f // P                       # input chunk count (2)
    D = hid_heads                         # 128
    H = HEADS
    wmax = int(max(Ks))

    nc = bacc.Bacc("TRN2", target_bir_lowering=False, debug=False,
                   enable_asserts=enable_asserts, num_devices=cores)

    GA = next(g for g in (7, 4, 2, 1) if wpc % g == 0)
    ngrp = wpc // GA
    xTt = nc.dram_tensor("xTt", [ngrp, P, GA * in_f], BF16,
                         kind="ExternalInput")
    w1cat = nc.dram_tensor("w1cat", [ic, P, D + 8], BF16, kind="ExternalInput")
    w2cat = nc.dram_tensor("w2cat", [D, D + 8], BF16, kind="ExternalInput")
    w3cat = nc.dram_tensor("w3cat", [D, out_f + 8], BF16, kind="ExternalInput")
    b1row = nc.dram_tensor("b1row", [1, D], F32, kind="ExternalInput")
    b2row = nc.dram_tensor("b2row", [1, D], F32, kind="ExternalInput")
    b3row = nc.dram_tensor("b3row", [1, out_f], F32, kind="ExternalInput")
    idxv = nc.dram_tensor("idxv", [P, S], I32, kind="ExternalInput")
    logew_d = nc.dram_tensor("logew", [P, S], BF16, kind="ExternalInput")
    out_d = nc.dram_tensor("out", [nc_rows, out_f], F32, kind="ExternalOutput")

    # internal DRAM
    h1tab = nc.dram_tensor("h1tab", [npad, D + H], BF16, addr_space="Shared")
    ag1h_in = nc.dram_tensor("ag1h_in", [nc_rows, D + H], BF16)
    ag2h_in = nc.dram_tensor("ag2h_in", [nc_rows, D + H], BF16)
    h2tab = nc.dram_tensor("h2tab", [npad, D + H], BF16, addr_space="Shared")
    ag3h_in = nc.dram_tensor("ag3h_in", [nc_rows, out_f + heads3], BF16)
    h3tab = nc.dram_tensor("h3tab", [npad, out_f + heads3], BF16,
                           addr_space="Shared")

    rg = [list(range(cores))]

    with TileContext(nc) as tc:
        with tc.tile_pool(name="const", bufs=1) as cp, \
             tc.tile_pool(name="xin", bufs=3) as xp, \
             tc.tile_pool(name="work", bufs=3) as wk, \
             tc.tile_pool(name="small", bufs=3) as sm, \
             tc.tile_pool(name="ps", bufs=3, space="PSUM") as pp, \
             tc.tile_pool(name="ps2", bufs=2, space="PSUM") as pp2:

            ident_b = cp.tile([P, P], BF16, tag="identb")
            make_identity(nc, ident_b[:])
            w1_sb = cp.tile([P, ic, D + 8], BF16, tag="w1")
            nc.sync.dma_start(out=w1_sb[:],
                              in_=w1cat.ap().rearrange("c p f -> p c f"))
            w2_sb = cp.tile([P, D + 8], BF16, tag="w2")
            nc.sync.dma_start(out=w2_sb[:], in_=w2cat[:, :])
            w3_sb = cp.tile([P, out_f + 8], BF16, tag="w3")
            nc.sync.dma_start(out=w3_sb[:], in_=w3cat[:, :])
            b1_sb = cp.tile([P, D], F32, tag="b1")
            nc.sync.dma_start(out=b1_sb[:], in_=b1row.ap().to_broadcast((P, D)))
            b2_sb = cp.tile([P, D], F32, tag="b2")
            nc.sync.dma_start(out=b2_sb[:], in_=b2row.ap().to_broadcast((P, D)))
            b3_sb = cp.tile([P, out_f], F32, tag="b3")
            nc.sync.dma_start(out=b3_sb[:],
                              in_=b3row.ap().to_broadcast((P, out_f)))
            idx_sb = cp.tile([P, S], I32, tag="idx")
            nc.sync.dma_start(out=idx_sb[:], in_=idxv[:, :])
            lew_sb = cp.tile([P, S], BF16, tag="lew")
            nc.sync.dma_start(out=lew_sb[:], in_=logew_d[:, :])

            # ---------------- stage A: layer-1 tables (sharded) ------------
            # Each core computes only its own wpc tiles; tables are then
            # AllGather'd. Own-window ald1 comes straight from the PSUM.
            ald1_all = cp.tile([P, wpc, H], BF16, tag="ald1all")
            for t0 in range(0, wpc, GA):
                xt = xp.tile([P, GA, ic, P], BF16, tag="xt")
                nc.sync.dma_start(out=xt[:], in_=xTt[t0 // GA])
                h_sb = sm.tile([P, GA, D + H], BF16, tag="hA")
                for g in range(GA):
                    ps = pp.tile([P, D + 8], F32, tag="acc")
                    for c in range(ic):
                        nc.tensor.matmul(ps[:], lhsT=xt[:, g, c, :],
                                         rhs=w1_sb[:, c, :],
                                         start=(c == 0), stop=(c == ic - 1))
                    nc.scalar.activation(h_sb[:, g, :], ps[:, 0:D + H],
                                         AF.Copy)
                    nc.vector.tensor_copy(out=ald1_all[:, t0 + g, :],
                                          in_=ps[:, D + 4:D + 4 + H])
                nc.sync.dma_start(
                    out=ag1h_in[t0 * P:(t0 + GA) * P, :].rearrange(
                        "(g p) f -> p g f", p=P),
                    in_=h_sb[:])
            nc.gpsimd.collective_compute(
                "AllGather", mybir.AluOpType.bypass, replica_groups=rg,
                ins=[ag1h_in.ap().opt()], outs=[h1tab.ap().opt()])
            ald2_all = cp.tile([P, wpc, H], BF16, tag="ald2all")
            ald3_all = cp.tile([P, wpc, 1], BF16, tag="ald3all")

            # ---------------- edge pass ------------------------------------
            def edge_layer(layer, tab, ald_all, F_in, H_l, wnext_sb,
                           F_next, H_n, bias_sb, agh, aldnext):
                """One GAT layer over all windows. ald_all: SBUF tile."""
                for w in range(wpc):
                    K = int(Ks[w])
                    off = int(offs[w])
                    r0 = w * P
                    V = wk.tile([P, K, F_in + H_l], BF16, tag="V")
                    for k in range(K):
                        nc.gpsimd.indirect_dma_start(
                            out=V[:, k, :], out_offset=None, in_=tab[:, :],
                            in_offset=bass.IndirectOffsetOnAxis(
                                ap=idx_sb[:, off + k:off + k + 1], axis=0))
                    ALS = V[:, :, F_in:F_in + H_l]
                    ald_w = ald_all[:, w, :]
                    # logit = ALS + ald (bcast over K) + logew (bcast over H)
                    logit = wk.tile([P, K, H_l], BF16, tag="logit")
                    ald_b = bass.AP(ald_w.tensor, ald_w.offset,
                                    [ald_w.ap[0], [0, K], [1, H_l]])
                    nc.vector.tensor_add(out=logit[:], in0=ALS, in1=ald_b)
                    lew_ap = lew_sb[:, off:off + K]
                    lew_b = bass.AP(lew_ap.tensor, lew_ap.offset,
                                    [lew_ap.ap[0], [1, K], [0, H_l]])
                    nc.vector.tensor_add(out=logit[:], in0=logit[:],
                                         in1=lew_b)
                    # w = exp(max(0.2*logit, logit)), rounded to bf16 on write
                    wt = wk.tile([P, K, H_l], BF16, tag="wt")
                    nc.vector.scalar_tensor_tensor(
                        out=wt[:], in0=logit[:], scalar=SLOPE, in1=logit[:],
                        op0=ALU.mult, op1=ALU.max)
                    wtb = wk.tile([P, K, H_l], BF16, tag="wtb")
                    nc.scalar.activation(wtb[:], wt[:], AF.Exp)
                    # rhs = [V*w | w]
                    rhs = wk.tile([P, K, F_in + H_l], BF16, tag="rhs")
                    ch = F_in // H_l
                    wrep = bass.AP(wtb.tensor, wtb[:].offset,
                                   [wtb[:].ap[0], [H_l, K], [1, H_l], [0, ch]])
                    nc.vector.tensor_mul(out=rhs[:, :, 0:F_in],
                                         in0=V[:, :, 0:F_in], in1=wrep)
                    nc.vector.tensor_copy(out=rhs[:, :, F_in:F_in + H_l],
                                          in_=wtb[:])
                    acc = pp.tile([P, F_in + H_l], F32, tag="acc")
                    for k in range(K):
                        nc.tensor.matmul(acc[:], lhsT=ident_b[:],
                                         rhs=rhs[:, k, :],
                                         start=(k == 0), stop=(k == K - 1))
                    den = sm.tile([P, H_l], F32, tag="den")
                    nc.vector.tensor_scalar_add(den[:],
                                                acc[:, F_in:F_in + H_l],
                                                1e-16)
                    rec = sm.tile([P, H_l], F32, tag="rec")
                    nc.vector.reciprocal(rec[:], den[:])
                    o = sm.tile([P, F_in], F32, tag="o")
                    rrep = bass.AP(rec.tensor, rec[:].offset,
                                   [rec[:].ap[0], [1, H_l], [0, ch]])
                    nc.vector.tensor_mul(out=o[:], in0=acc[:, 0:F_in],
                                         in1=rrep)
                    nc.vector.tensor_add(out=o[:], in0=o[:], in1=bias_sb[:])
                    if layer < 3:
                        o_b = sm.tile([P, F_in], BF16, tag="ob")
                        nc.scalar.activation(o_b[:], o[:], AF.Relu)
                        # next-layer table rows for this window
                        oT = pp2.tile([P, P], BF16, tag="oT")
                        nc.tensor.transpose(out=oT[:], in_=o_b[:],
                                            identity=ident_b[:])
                        oT_sb = sm.tile([P, P], BF16, tag="oTsb")
                        nc.scalar.activation(oT_sb[:], oT[:], AF.Copy)
                        hn = pp2.tile([P, F_next + 8], F32, tag="hn")
                        nc.tensor.matmul(hn[:], lhsT=oT_sb[:],
                                         rhs=wnext_sb[:, 0:F_next + 8],
                                         start=True, stop=True)
                        hn_sb = sm.tile([P, F_next + H_n], BF16, tag="hnsb")
                        nc.scalar.activation(hn_sb[:, 0:F_next],
                                             hn[:, 0:F_next], AF.Copy)
                        nc.vector.tensor_copy(
                            out=hn_sb[:, F_next:F_next + H_n],
                            in_=hn[:, F_next:F_next + H_n])
                        nc.sync.dma_start(out=agh[r0:r0 + P, :], in_=hn_sb[:])
                        nc.vector.tensor_copy(
                            out=aldnext[:, w, :],
                            in_=hn[:, F_next + 4:F_next + 4 + H_n])
                    else:
                        nc.sync.dma_start(out=out_d[r0:r0 + P, :], in_=o[:])

            # layer 1
            edge_layer(1, h1tab, ald1_all, D, H,
                       w2_sb, D, H, b1_sb, ag2h_in, ald2_all)
            nc.gpsimd.collective_compute(
                "AllGather", mybir.AluOpType.bypass, replica_groups=rg,
                ins=[ag2h_in.ap().opt()], outs=[h2tab.ap().opt()])
            # layer 2
            edge_layer(2, h2tab, ald2_all, D, H,
                       w3_sb, out_f, heads3, b2_sb, ag3h_in, ald3_all)
            nc.gpsimd.collective_compute(
                "AllGather", mybir.AluOpType.bypass, replica_groups=rg,
                ins=[ag3h_in.ap().opt()], outs=[h3tab.ap().opt()])
            # layer 3
            edge_layer(3, h3tab, ald3_all, out_f, heads3,
                       None, 0, 1, b3_sb, None, None)
    nc.finalize()
    return nc


# ----------------------------------------------------------------------------
# host entry point
# ----------------------------------------------------------------------------
def prepare_inputs(x, edge_index, edge_weight, W1, a_src1, a_dst1, b1,
                   W2, a_src2, a_dst2, b2, W3, a_src3, a_dst3, b3,
                   npad, cores):
    """Returns (in_maps, perm, Ks, offs, S)."""
    x = np.asarray(x, np.float32)
    W1 = np.asarray(W1, np.float32)
    W2 = np.asarray(W2, np.float32)
    W3 = np.asarray(W3, np.float32)
    n_nodes, in_f = x.shape
    d1 = W1.shape[1]
    out_f = W3.shape[1]
    heads = np.asarray(a_src1).shape[0]
    hid = d1 // heads

    perm, Ks, offs, gidx, logew = build_schedule(
        edge_index[0], edge_index[1], edge_weight, n_nodes, npad, cores)

    xp = np.zeros((npad, in_f), np.float32)
    xp[perm[:n_nodes]] = x
    nc_rows_ = npad // cores
    wpc_ = nc_rows_ // P
    GA = next(g for g in (7, 4, 2, 1) if wpc_ % g == 0)
    ic = in_f // P
    # per-core transposed x: [core, grp, p, g*c*n]
    A = xp.reshape(cores, wpc_ // GA, GA, P, ic, P)   # [c, grp, g, n, ch, p]
    xTt_pc = _np_bf16(np.ascontiguousarray(
        A.transpose(0, 1, 5, 2, 4, 3)).reshape(cores, wpc_ // GA, P,
                                               GA * in_f))

    def wcat(W, a_s, a_d, h, c):
        wa = (W.reshape(W.shape[0], h, c) * np.asarray(a_s)[None]).sum(-1)
        wd = (W.reshape(W.shape[0], h, c) * np.asarray(a_d)[None]).sum(-1)
        pad = np.zeros((W.shape[0], 4 - wa.shape[1]), np.float32)
        return np.concatenate([W, wa, pad, wd, pad], axis=1)

    w1full = wcat(W1, a_src1, a_dst1, heads, hid)          # [256, 136]
    w1cat = _np_bf16(w1full.reshape(2, P, d1 + 8))
    w2cat = _np_bf16(wcat(W2, a_src2, a_dst2, heads, hid))  # [128, 136]
    w3cat = _np_bf16(wcat(W3, a_src3, a_dst3, 1, out_f))    # [128, 72]

    in_maps = []
    for c in range(cores):
        in_maps.append(dict(
            xTt=xTt_pc[c], w1cat=w1cat, w2cat=w2cat, w3cat=w3cat,
            b1row=np.asarray(b1, np.float32).reshape(1, -1),
            b2row=np.asarray(b2, np.float32).reshape(1, -1),
            b3row=np.asarray(b3, np.float32).reshape(1, -1),
            idxv=gidx[c], logew=_np_bf16(logew[c]),
        ))
    return in_maps, perm, Ks, offs


def kernel(**inputs):
    npad = 50176
    in_maps, perm, Ks, offs = prepare_inputs(
        npad=npad, cores=CORES, **inputs)
    S = int(offs[-1])
    nc = build_program(npad, Ks, offs, S, IN, HEADS * HID, OUT, 1, CORES)

    from concourse.bass_utils import run_bass_kernel_spmd
    res = run_bass_kernel_spmd(nc, in_maps, core_ids=list(range(CORES)))
    shards = [res.results[c]["out"] for c in range(CORES)]
    full = np.concatenate(shards, axis=0)       # [npad, OUT] in new-id order
    return full[perm[:N]].astype(np.float32)



# revision 31
# speedup vs baseline: 1.4423x; 1.0035x over previous
"""3-layer GAT on 8 TRN2 NeuronCores.

Strategy (1D vertex-cut, dst-sharded):
  * Nodes are permuted: degree-sorted, dealt round-robin to 8 cores, so each
    core owns a contiguous range of NC=6272 "new" node ids whose windows of
    128 consecutive ids have near-uniform in-degree.
  * Per (core, window) the edge list is laid out as K[w] "slots" x 128 dst
    rows (shared K schedule across cores, padding slots have log_ew=-1e30).
  * Per layer, per window: indirect-DMA gather of h[src] rows (bf16) and
    als[src] (f32), attention weight w = exp(leaky(als+ald)+log_ew),
    rhs = [V*w | w], K identity-lhsT matmuls accumulate [128, F+H] in PSUM
    (weighted segment-sum + denominator), normalize, bias(+ReLU).
  * Next-layer tables h'=relu(o)@W', als'=o@(W'·a) are produced per window
    (PE transpose + one matmul) and AllGather'd across cores between layers.
  * segment_max is omitted: logits are bounded (|logit| < ~8), exp is safe
    in f32, and softmax is shift-invariant, so this is mathematically
    identical to the reference.
"""
import numpy as np

# problem constants (hardcoded per harness contract)
N, E, IN, HID, HEADS, OUT = 50000, 800000, 256, 32, 4, 64
SLOPE = 0.2
CORES = 8
P = 128


# ----------------------------------------------------------------------------
# host-side schedule construction (index data only)
# ----------------------------------------------------------------------------
def build_schedule(src, dst, ew, n_nodes, npad, cores):
    """Returns node permutation + per-core slot arrays.

    perm: old->new node id (len npad); Ks: [W] slots per window;
    gidx: [cores, 128, S] int32 src new-ids; logew: [cores, 128, S] f32.
    """
    nc_rows = npad // cores
    wpc = nc_rows // P
    src = np.asarray(src, np.int64)
    dst = np.asarray(dst, np.int64)
    ew = np.asarray(ew, np.float32)

    deg = np.bincount(dst, minlength=npad)
    order = np.argsort(-deg, kind="stable")          # ranks -> old id
    perm = np.empty(npad, np.int64)
    ranks = np.arange(npad)
    perm[order] = (ranks % cores) * nc_rows + ranks // cores

    nsrc = perm[src]
    ndst = perm[dst]
    eorder = np.argsort(ndst, kind="stable")
    nsrc_s = nsrc[eorder].astype(np.int32)
    ndst_s = ndst[eorder]
    ew_s = ew[eorder]

    counts = np.bincount(ndst_s, minlength=npad)
    starts = np.zeros(npad + 1, np.int64)
    np.cumsum(counts, out=starts[1:])
    rank_in_dst = np.arange(len(ndst_s)) - starts[ndst_s]

    Ks = counts.reshape(cores, wpc, P).max(axis=(0, 2))
    Ks = np.maximum(Ks, 1).astype(np.int64)
    offs = np.zeros(wpc + 1, np.int64)
    np.cumsum(Ks, out=offs[1:])
    S = int(offs[-1])

    core_e = ndst_s // nc_rows
    loc = ndst_s % nc_rows
    w_e = loc // P
    p_e = loc % P
    col = offs[w_e] + rank_in_dst

    gidx = np.zeros((cores, P, S), np.int32)
    logew = np.full((cores, P, S), -1e30, np.float32)
    flat = (core_e * P + p_e) * S + col
    gidx.reshape(-1)[flat] = nsrc_s
    with np.errstate(divide="ignore"):
        logew.reshape(-1)[flat] = np.log(np.maximum(ew_s, 0.0)).astype(np.float32)
    return perm, Ks, offs, gidx, logew


def _np_bf16(x):
    import ml_dtypes
    return np.asarray(x, np.float32).astype(ml_dtypes.bfloat16)


# ----------------------------------------------------------------------------
# device program
# ----------------------------------------------------------------------------
def build_program(npad, Ks, offs, S, in_f, hid_heads, out_f, heads3, cores,
                  enable_asserts=False, debug_taps=False):
    """Build the SPMD Bacc program. Shapes:
      xTt   [NT, in_f, 128] bf16   (transposed x, node-tile blocks)
      w1cat [in_f//128, 128, hid_heads+8] bf16
      w2cat [hid_heads, hid_heads+8] bf16
      w3cat [hid_heads, out_f+2] bf16
      b1row/b2row [1, hid_heads] f32 ; b3row [1, out_f] f32
      idxv  [128, S] int32 ; logew [128, S] f32 ; idx_ald [128, WPC] int32
      out   [NC, out_f] f32 (per-core shard)
    """
    import concourse.bacc as bacc
    import concourse.bass as bass
    import concourse.mybir as mybir
    from concourse.masks import make_identity
    from concourse.tile import TileContext

    F32, BF16, I32 = mybir.dt.float32, mybir.dt.bfloat16, mybir.dt.int32
    AF = mybir.ActivationFunctionType
    ALU = mybir.AluOpType

    nc_rows = npad // cores
    wpc = nc_rows // P
    nt = npad // P
    ic = in_f // P                       # input chunk count (2)
    D = hid_heads                         # 128
    H = HEADS
    wmax = int(max(Ks))

    nc = bacc.Bacc("TRN2", target_bir_lowering=False, debug=False,
                   enable_asserts=enable_asserts, num_devices=cores)

    GA = next(g for g in (7, 4, 2, 1) if wpc % g == 0)
    ngrp = wpc // GA
    xTt = nc.dram_tensor("xTt", [ngrp, P, GA * in_f], BF16,
                         kind="ExternalInput")
    w1cat = nc.dram_tensor("w1cat", [ic, P, D + 8], BF16, kind="ExternalInput")
    w2cat = nc.dram_tensor("w2cat", [D, D + 8], BF16, kind="ExternalInput")
    w3cat = nc.dram_tensor("w3cat", [D, out_f + 8], BF16, kind="ExternalInput")
    b1row = nc.dram_tensor("b1row", [1, D], F32, kind="ExternalInput")
    b2row = nc.dram_tensor("b2row", [1, D], F32, kind="ExternalInput")
    b3row = nc.dram_tensor("b3row", [1, out_f], F32, kind="ExternalInput")
    idxv = nc.dram_tensor("idxv", [P, S], I32, kind="ExternalInput")
    logew_d = nc.dram_tensor("logew", [P, S], BF16, kind="ExternalInput")
    out_d = nc.dram_tensor("out", [nc_rows, out_f], F32, kind="ExternalOutput")

    # internal DRAM
    h1tab = nc.dram_tensor("h1tab", [npad, D + H], BF16, addr_space="Shared")
    ag1h_in = nc.dram_tensor("ag1h_in", [nc_rows, D + H], BF16)
    ag2h_in = nc.dram_tensor("ag2h_in", [nc_rows, D + H], BF16)
    h2tab = nc.dram_tensor("h2tab", [npad, D + H], BF16, addr_space="Shared")
    ag3h_in = nc.dram_tensor("ag3h_in", [nc_rows, out_f + heads3], BF16)
    h3tab = nc.dram_tensor("h3tab", [npad, out_f + heads3], BF16,
                           addr_space="Shared")

    rg = [list(range(cores))]

    with TileContext(nc) as tc:
        with tc.tile_pool(name="const", bufs=1) as cp, \
             tc.tile_pool(name="xin", bufs=3) as xp, \
             tc.tile_pool(name="work", bufs=4) as wk, \
             tc.tile_pool(name="small", bufs=4) as sm, \
             tc.tile_pool(name="ps", bufs=3, space="PSUM") as pp, \
             tc.tile_pool(name="ps2", bufs=2, space="PSUM") as pp2:

            ident_b = cp.tile([P, P], BF16, tag="identb")
            make_identity(nc, ident_b[:])
            w1_sb = cp.tile([P, ic, D + 8], BF16, tag="w1")
            nc.sync.dma_start(out=w1_sb[:],
                              in_=w1cat.ap().rearrange("c p f -> p c f"))
            w2_sb = cp.tile([P, D + 8], BF16, tag="w2")
            nc.sync.dma_start(out=w2_sb[:], in_=w2cat[:, :])
            w3_sb = cp.tile([P, out_f + 8], BF16, tag="w3")
            nc.sync.dma_start(out=w3_sb[:], in_=w3cat[:, :])
            b1_sb = cp.tile([P, D], F32, tag="b1")
            nc.sync.dma_start(out=b1_sb[:], in_=b1row.ap().to_broadcast((P, D)))
            b2_sb = cp.tile([P, D], F32, tag="b2")
            nc.sync.dma_start(out=b2_sb[:], in_=b2row.ap().to_broadcast((P, D)))
            b3_sb = cp.tile([P, out_f], F32, tag="b3")
            nc.sync.dma_start(out=b3_sb[:],
                              in_=b3row.ap().to_broadcast((P, out_f)))
            idx_sb = cp.tile([P, S], I32, tag="idx")
            nc.sync.dma_start(out=idx_sb[:], in_=idxv[:, :])
            lew_sb = cp.tile([P, S], BF16, tag="lew")
            nc.sync.dma_start(out=lew_sb[:], in_=logew_d[:, :])

            # ---------------- stage A: layer-1 tables (sharded) ------------
            # Each core computes only its own wpc tiles; tables are then
            # AllGather'd. Own-window ald1 comes straight from the PSUM.
            ald1_all = cp.tile([P, wpc, H], BF16, tag="ald1all")
            for t0 in range(0, wpc, GA):
                xt = xp.tile([P, GA, ic, P], BF16, tag="xt")
                nc.sync.dma_start(out=xt[:], in_=xTt[t0 // GA])
                h_sb = sm.tile([P, GA, D + H], BF16, tag="hA")
                for g in range(GA):
                    ps = pp.tile([P, D + 8], F32, tag="acc")
                    for c in range(ic):
                        nc.tensor.matmul(ps[:], lhsT=xt[:, g, c, :],
                                         rhs=w1_sb[:, c, :],
                                         start=(c == 0), stop=(c == ic - 1))
                    nc.scalar.activation(h_sb[:, g, :], ps[:, 0:D + H],
                                         AF.Copy)
                    nc.vector.tensor_copy(out=ald1_all[:, t0 + g, :],
                                          in_=ps[:, D + 4:D + 4 + H])
                nc.sync.dma_start(
                    out=ag1h_in[t0 * P:(t0 + GA) * P, :].rearrange(
                        "(g p) f -> p g f", p=P),
                    in_=h_sb[:])
            nc.gpsimd.collective_compute(
                "AllGather", mybir.AluOpType.bypass, replica_groups=rg,
                ins=[ag1h_in.ap().opt()], outs=[h1tab.ap().opt()])
            ald2_all = cp.tile([P, wpc, H], BF16, tag="ald2all")
            ald3_all = cp.tile([P, wpc, 1], BF16, tag="ald3all")

            # ---------------- edge pass ------------------------------------
            def edge_layer(layer, tab, ald_all, F_in, H_l, wnext_sb,
                           F_next, H_n, bias_sb, agh, aldnext):
                """One GAT layer over all windows. ald_all: SBUF tile."""
                for w in range(wpc):
                    K = int(Ks[w])
                    off = int(offs[w])
                    r0 = w * P
                    V = wk.tile([P, K, F_in + H_l], BF16, tag="V")
                    for k in range(K):
                        nc.gpsimd.indirect_dma_start(
                            out=V[:, k, :], out_offset=None, in_=tab[:, :],
                            in_offset=bass.IndirectOffsetOnAxis(
                                ap=idx_sb[:, off + k:off + k + 1], axis=0))
                    ALS = V[:, :, F_in:F_in + H_l]
                    ald_w = ald_all[:, w, :]
                    # logit = ALS + ald (bcast over K) + logew (bcast over H)
                    logit = wk.tile([P, K, H_l], BF16, tag="logit")
                    ald_b = bass.AP(ald_w.tensor, ald_w.offset,
                                    [ald_w.ap[0], [0, K], [1, H_l]])
                    nc.vector.tensor_add(out=logit[:], in0=ALS, in1=ald_b)
                    lew_ap = lew_sb[:, off:off + K]
                    lew_b = bass.AP(lew_ap.tensor, lew_ap.offset,
                                    [lew_ap.ap[0], [1, K], [0, H_l]])
                    nc.vector.tensor_add(out=logit[:], in0=logit[:],
                                         in1=lew_b)
                    # w = exp(max(0.2*logit, logit)), rounded to bf16 on write
                    wt = wk.tile([P, K, H_l], BF16, tag="wt")
                    nc.vector.scalar_tensor_tensor(
                        out=wt[:], in0=logit[:], scalar=SLOPE, in1=logit[:],
                        op0=ALU.mult, op1=ALU.max)
                    wtb = wk.tile([P, K, H_l], BF16, tag="wtb")
                    nc.scalar.activation(wtb[:], wt[:], AF.Exp)
                    # rhs = [V*w | w]
                    rhs = wk.tile([P, K, F_in + H_l], BF16, tag="rhs")
                    ch = F_in // H_l
                    wrep = bass.AP(wtb.tensor, wtb[:].offset,
                                   [wtb[:].ap[0], [H_l, K], [1, H_l], [0, ch]])
                    nc.vector.tensor_mul(out=rhs[:, :, 0:F_in],
                                         in0=V[:, :, 0:F_in], in1=wrep)
                    nc.vector.tensor_copy(out=rhs[:, :, F_in:F_in + H_l],
                                          in_=wtb[:])
                    acc = pp.tile([P, F_in + H_l], F32, tag="acc")
                    for k in range(K):
                        nc.tensor.matmul(acc[:], lhsT=ident_b[:],
                                         rhs=rhs[:, k, :],
                                         start=(k == 0), stop=(k == K - 1))
                    den = sm.tile([P, H_l], F32, tag="den")
                    nc.vector.tensor_scalar_add(den[:],
                                                acc[:, F_in:F_in + H_l],
                                                1e-16)
                    rec = sm.tile([P, H_l], F32, tag="rec")
                    nc.vector.reciprocal(rec[:], den[:])
                    o = sm.tile([P, F_in], F32, tag="o")
                    rrep = bass.AP(rec.tensor, rec[:].offset,
                                   [rec[:].ap[0], [1, H_l], [0, ch]])
                    nc.vector.tensor_mul(out=o[:], in0=acc[:, 0:F_in],
                                         in1=rrep)
                    nc.vector.tensor_add(out=o[:], in0=o[:], in1=bias_sb[:])
                    if layer < 3:
                        o_b = sm.tile([P, F_in], BF16, tag="ob")
                        nc.scalar.activation(o_b[:], o[:], AF.Relu)
                        # next-layer table rows for this window
                        oT = pp2.tile([P, P], BF16, tag="oT")
                        nc.tensor.transpose(out=oT[:], in_=o_b[:],
                                            identity=ident_b[:])
                        oT_sb = sm.tile([P, P], BF16, tag="oTsb")
                        nc.scalar.activation(oT_sb[:], oT[:], AF.Copy)
                        hn = pp2.tile([P, F_next + 8], F32, tag="hn")
                        nc.tensor.matmul(hn[:], lhsT=oT_sb[:],
                                         rhs=wnext_sb[:, 0:F_next + 8],
                                         start=True, stop=True)
                        hn_sb = sm.tile([P, F_next + H_n], BF16, tag="hnsb")
                        nc.scalar.activation(hn_sb[:, 0:F_next],
                                             hn[:, 0:F_next], AF.Copy)
                        nc.vector.tensor_copy(
                            out=hn_sb[:, F_next:F_next + H_n],
                            in_=hn[:, F_next:F_next + H_n])
                        nc.sync.dma_start(out=agh[r0:r0 + P, :], in_=hn_sb[:])
                        nc.vector.tensor_copy(
                            out=aldnext[:, w, :],
                            in_=hn[:, F_next + 4:F_next + 4 + H_n])
                    else:
                        nc.sync.dma_start(out=out_d[r0:r0 + P, :], in_=o[:])

            # layer 1
            edge_layer(1, h1tab, ald1_all, D, H,
                       w2_sb, D, H, b1_sb, ag2h_in, ald2_all)
            nc.gpsimd.collective_compute(
                "AllGather", mybir.AluOpType.bypass, replica_groups=rg,
                ins=[ag2h_in.ap().opt()], outs=[h2tab.ap().opt()])
            # layer 2
            edge_layer(2, h2tab, ald2_all, D, H,
                       w3_sb, out_f, heads3, b2_sb, ag3h_in, ald3_all)
            nc.gpsimd.collective_compute(
                "AllGather", mybir.AluOpType.bypass, replica_groups=rg,
                ins=[ag3h_in.ap().opt()], outs=[h3tab.ap().opt()])
            # layer 3
            edge_layer(3, h3tab, ald3_all, out_f, heads3,
                       None, 0, 1, b3_sb, None, None)
    nc.finalize()
    return nc


# ----------------------------------------------------------------------------
# host entry point
# ----------------------------------------------------------------------------
def prepare_inputs(x, edge_index, edge_weight, W1, a_src1, a_dst1, b1,
                   W2, a_src2, a_dst2, b2, W3, a_src3, a_dst3, b3,
                   npad, cores):
    """Returns (in_maps, perm, Ks, offs, S)."""
    x = np.asarray(x, np.float32)
    W1 = np.asarray(W1, np.float32)
    W2 = np.asarray(W2, np.float32)
    W3 = np.asarray(W3, np.float32)
    n_nodes, in_f = x.shape
    d1 = W1.shape[1]
    out_f = W3.shape[1]
    heads = np.asarray(a_src1).shape[0]
    hid = d1 // heads

    perm, Ks, offs, gidx, logew = build_schedule(
        edge_index[0], edge_index[1], edge_weight, n_nodes, npad, cores)

    xp = np.zeros((npad, in_f), np.float32)
    xp[perm[:n_nodes]] = x
    nc_rows_ = npad // cores
    wpc_ = nc_rows_ // P
    GA = next(g for g in (7, 4, 2, 1) if wpc_ % g == 0)
    ic = in_f // P
    # per-core transposed x: [core, grp, p, g*c*n]
    A = xp.reshape(cores, wpc_ // GA, GA, P, ic, P)   # [c, grp, g, n, ch, p]
    xTt_pc = _np_bf16(np.ascontiguousarray(
        A.transpose(0, 1, 5, 2, 4, 3)).reshape(cores, wpc_ // GA, P,
                                               GA * in_f))

    def wcat(W, a_s, a_d, h, c):
        wa = (W.reshape(W.shape[0], h, c) * np.asarray(a_s)[None]).sum(-1)
        wd = (W.reshape(W.shape[0], h, c) * np.asarray(a_d)[None]).sum(-1)
        pad = np.zeros((W.shape[0], 4 - wa.shape[1]), np.float32)
        return np.concatenate([W, wa, pad, wd, pad], axis=1)

    w1full = wcat(W1, a_src1, a_dst1, heads, hid)          # [256, 136]
    w1cat = _np_bf16(w1full.reshape(2, P, d1 + 8))
    w2cat = _np_bf16(wcat(W2, a_src2, a_dst2, heads, hid))  # [128, 136]
    w3cat = _np_bf16(wcat(W3, a_src3, a_dst3, 1, out_f))    # [128, 72]

    in_maps = []
    for c in range(cores):
        in_maps.append(dict(
            xTt=xTt_pc[c], w1cat=w1cat, w2cat=w2cat, w3cat=w3cat,
            b1row=np.asarray(b1, np.float32).reshape(1, -1),
            b2row=np.asarray(b2, np.float32).reshape(1, -1),
            b3row=np.asarray(b3, np.float32).reshape(1, -1),
            idxv=gidx[c], logew=_np_bf16(logew[c]),
        ))
    return in_maps, perm, Ks, offs


def kernel(**inputs):
    npad = 50176
    in_maps, perm, Ks, offs = prepare_inputs(
        npad=npad, cores=CORES, **inputs)
    S = int(offs[-1])
    nc = build_program(npad, Ks, offs, S, IN, HEADS * HID, OUT, 1, CORES)

    from concourse.bass_utils import run_bass_kernel_spmd
    res = run_bass_kernel_spmd(nc, in_maps, core_ids=list(range(CORES)))
    shards = [res.results[c]["out"] for c in range(CORES)]
    full = np.concatenate(shards, axis=0)       # [npad, OUT] in new-id order
    return full[perm[:N]].astype(np.float32)

